# revision 43
# baseline (speedup 1.0000x reference)
# Bass/Trainium2 kernel for nn_CPRPackedLinear (mixed 6-bit/5-bit packed
# quantized linear), tensor-parallel over out_features on 8 NeuronCores.
#
# Math: out = x_perm[:, :1024] @ deq6(W_high) + x_perm[:, 1024:] @ deq5(W_low) + bias
# with deq = (unpack_bits(packed) - half) * group_scale.
#
# v2 layout ("pair packing"): the host repacks quantized values into int16
# words holding TWO same-group values (v0 | v1<<B).  Partitions are
# group-exclusive per batch, so ONE [128, N] scale tile serves every word-row
# of the batch and all DVE ops are fully batched:
#   raw plane   = words * scale          (one batched 2x tensor_tensor)
#   masked ints = words & mask           (one batched 4x tensor_scalar)
#   masked plane= masked * scale         (one batched 2x tensor_tensor)
# Value recovery is linear, folded into the host-built xdup stationary rows:
#   v1 via raw/2^B,  v0 via masked + raw cancellation.
# 32 plane rows total (= info-theoretic minimum: 4096 k / 128 partitions).
# The (-half*scale) offsets and bias enter via one small correction matmul.
import numpy as np
import ml_dtypes

import concourse.bass as bass
import concourse.mybir as mybir
from concourse import bacc
from concourse.tile import TileContext
from concourse.bass_utils import run_bass_kernel_spmd

BF16 = ml_dtypes.bfloat16

N_CORES = 8
M = 64
OUT_FEATURES = 11008
N_PER = OUT_FEATURES // N_CORES  # 1376
N_HIGH = 1024
N_LOW = 3072
GROUP = 128

NCHUNKS = [(0, 512), (512, 512), (1024, 352)]

# batches: (region, word_rows, scale_tile_idx, mask, shift)
BATCHES = [("high", 4, 0, 63, 6), ("low1", 8, 1, 31, 5), ("low2", 4, 2, 31, 5)]
N_BLOCKS = 32

_P = np.arange(128)


def _kmaps():
    """k0/k1 index maps (within-region k) for each batch, [128, R]."""
    p = _P[:, None]
    r4 = np.arange(4)[None, :]
    r8 = np.arange(8)[None, :]
    k0h = 128 * (p // 16) + (p % 16) * 8 + 2 * r4
    k0l1 = 128 * (p // 8) + (p % 8) * 16 + 2 * r8
    k0l2 = 128 * (16 + p // 16) + (p % 16) * 8 + 2 * r4
    return {"high": k0h, "low1": k0l1, "low2": k0l2}


KMAP = _kmaps()


def build_nc():
    nc = bacc.Bacc(None, name="cpr_v2", enable_partition_id=False)
    f32 = mybir.dt.float32
    bf16 = mybir.dt.bfloat16
    i16 = mybir.dt.int16

    wh = nc.dram_tensor("wh", [128, 4, N_PER], i16, kind="ExternalInput")
    wl = nc.dram_tensor("wl", [128, 12, N_PER], i16, kind="ExternalInput")
    st = nc.dram_tensor("st", [3, 128, N_PER], bf16, kind="ExternalInput")
    xd = nc.dram_tensor("xdup", [128, N_BLOCKS, M], bf16, kind="ExternalInput")
    cl = nc.dram_tensor("corr_lhsT", [33, M], f32, kind="ExternalInput")
    cr = nc.dram_tensor("corr_rhs", [33, N_PER], f32, kind="ExternalInput")
    out = nc.dram_tensor("out", [M, N_PER], bf16, kind="ExternalOutput")

    with TileContext(nc) as tc, \
         tc.tile_pool(name="const", bufs=1) as const_pool, \
         tc.tile_pool(name="words", bufs=1) as words_pool, \
         tc.tile_pool(name="planes", bufs=2) as plane_pool, \
         tc.tile_pool(name="psum", bufs=1, space="PSUM") as psum_pool:

        # small tensors + first batch's data first; the very first weight row
        # and scale tile are split at the chunk-0 boundary so the first
        # dequant op + matmul only wait on 128KB per DMA queue
        st_t = const_pool.tile([128, 3, N_PER], bf16, tag="st")
        nc.scalar.dma_start(out=st_t[:, 0, 0:512], in_=st[0][:, 0:512])
        nc.scalar.dma_start(out=st_t[:, 0, 512:], in_=st[0][:, 512:])
        xd_t = const_pool.tile([128, N_BLOCKS, M], bf16, tag="xd")
        nc.scalar.dma_start(out=xd_t[:], in_=xd[:])
        wh_t = words_pool.tile([128, 4, N_PER], i16, tag="wh")
        nc.sync.dma_start(out=wh_t[:, 0:1, 0:512], in_=wh[:, 0:1, 0:512])
        nc.sync.dma_start(out=wh_t[:, 0:1, 512:], in_=wh[:, 0:1, 512:])
        nc.sync.dma_start(out=wh_t[:, 1:2], in_=wh[:, 1:2])
        nc.sync.dma_start(out=wh_t[:, 2:4], in_=wh[:, 2:4])
        cl_t = const_pool.tile([33, M], f32, tag="cl")
        nc.scalar.dma_start(out=cl_t[:], in_=cl[:])
        cr_t = const_pool.tile([33, N_PER], f32, tag="cr")
        nc.scalar.dma_start(out=cr_t[:], in_=cr[:])
        nc.scalar.dma_start(out=st_t[:, 1], in_=st[1])
        wl_t = words_pool.tile([128, 12, N_PER], i16, tag="wl")
        for i in range(0, 8, 2):
            nc.sync.dma_start(out=wl_t[:, i:i + 2], in_=wl[:, i:i + 2])
        nc.scalar.dma_start(out=st_t[:, 2], in_=st[2])
        nc.sync.dma_start(out=wl_t[:, 8:10], in_=wl[:, 8:10])
        nc.sync.dma_start(out=wl_t[:, 10:12], in_=wl[:, 10:12])

        out_sb = const_pool.tile([M, N_PER], bf16, tag="outsb")
        psums = [psum_pool.tile([M, w], mybir.dt.float32, tag=f"ps{i}",
                                name=f"ps{i}")
                 for i, (o, w) in enumerate(NCHUNKS)]

        def mm(bi, plane_ap, start=False, stop=False, skip=False):
            for ci, (o, w) in enumerate(NCHUNKS):
                nc.tensor.matmul(
                    psums[ci][:, :w], xd_t[:, bi, :], plane_ap[:, o:o + w],
                    start=start, stop=stop, skip_group_check=skip,
                )
            if start:
                # correction accumulates early (order-free in PSUM)
                for ci, (o, w) in enumerate(NCHUNKS):
                    nc.tensor.matmul(
                        psums[ci][:, :w], cl_t[:], cr_t[:, o:o + w],
                        start=False, stop=False, skip_group_check=True,
                    )

        # rows per DVE op: small first piece starts the PE early, bigger
        # pieces amortize per-op overhead afterwards.  The last GP_OFF rows
        # of the high-masked and low2-masked blocks run on GPSIMD (slower,
        # but in parallel with the DVE); their matmuls are issued late so
        # the strict-FIFO PE queue never blocks waiting on GPSIMD.
        # raw high pieces cover rows 1..3 (row 0 is emitted column-split);
        # masked blocks use MPIECES (all rows)
        PIECES = {"high": [1, 2], "low1": [4, 4], "low2": [2, 1, 1]}
        MPIECES = {"high": [2, 2], "low1": [4, 4], "low2": [2, 1, 1]}
        GP_OFF = 0
        BASE = {"high": 0, "low1": 8, "low2": 24}

        tiles = {}
        for (region, R, si, mask, shift) in BATCHES:
            if region == "high":
                words = wh_t[:]
            elif region == "low1":
                words = wl_t[:, 0:8]
            else:
                words = wl_t[:, 8:12]
            tiles[region] = dict(
                words=words, si=si,
                raw=plane_pool.tile([128, R, N_PER], bf16, tag="raw",
                                    name=f"raw{si}", bufs=2),
                mski=plane_pool.tile([128, R, N_PER], i16, tag="mski",
                                     name=f"mski{si}", bufs=2),
                mskp=plane_pool.tile([128, R, N_PER], bf16, tag="mskp",
                                     name=f"mskp{si}", bufs=2),
                gpp=(plane_pool.tile([128, GP_OFF, N_PER], bf16, tag="gpp",
                                     name=f"gpp{si}", bufs=2)
                     if GP_OFF else None),
            )

        def stb(si, rows):
            return st_t[:, si].rearrange("p (o n) -> p o n", o=1) \
                              .broadcast_to([128, rows, N_PER])

        def emit_first_row():
            """High row 0, column-split at the chunk-0 edge: the first matmul
            (chunk 0, start + correction) launches off a quarter of the row."""
            t = tiles["high"]
            for (c0, c1) in [(0, 512), (512, N_PER)]:
                nc.vector.tensor_tensor(
                    t["raw"][:, 0:1, c0:c1], t["words"][:, 0:1, c0:c1],
                    st_t[:, 0, c0:c1].rearrange("p (o n) -> p o n", o=1),
                    mybir.AluOpType.mult)
                for ci, (o, w) in enumerate(NCHUNKS):
                    if not (c0 <= o < c1):
                        continue
                    nc.tensor.matmul(
                        psums[ci][:, :w], xd_t[:, 0, :], t["raw"][:, 0, o:o + w],
                        start=True, stop=False)
                    nc.tensor.matmul(
                        psums[ci][:, :w], cl_t[:], cr_t[:, o:o + w],
                        start=False, stop=False, skip_group_check=True)

        def emit_raw(region, R, si, skip_first=False):
            t = tiles[region]
            r0 = 1 if skip_first else 0
            for pc in PIECES[region]:
                sl_ = slice(r0, r0 + pc)
                nc.vector.tensor_tensor(t["raw"][:, sl_], t["words"][:, sl_],
                                        stb(si, pc), mybir.AluOpType.mult)
                for r in range(r0, r0 + pc):
                    mm(BASE[region] + r, t["raw"][:, r])
                r0 += pc

        def emit_masked(region, R, si, mask, gp_rows=0, mm_skip_gp=True,
                        stop_last=False):
            """DVE part of the masked block; last gp_rows rows go to GPSIMD
            (compute only - their matmuls are emitted by emit_gp_mms)."""
            t = tiles[region]
            ndve = R - gp_rows
            r0 = 0
            for pc in MPIECES[region]:
                sl_ = slice(r0, r0 + pc)
                nc.vector.tensor_scalar(t["mski"][:, sl_], t["words"][:, sl_],
                                        mask, None,
                                        mybir.AluOpType.bitwise_and)
                if r0 < ndve:
                    dv = slice(r0, min(r0 + pc, ndve))
                    nc.vector.tensor_tensor(t["mskp"][:, dv], t["mski"][:, dv],
                                            stb(si, dv.stop - dv.start),
                                            mybir.AluOpType.mult)
                    for r in range(dv.start, dv.stop):
                        last = stop_last and gp_rows == 0 and r == R - 1
                        mm(BASE[region] + R + r, t["mskp"][:, r], stop=last)
                r0 += pc
            for r in range(ndve, R):
                nc.gpsimd.tensor_tensor(t["gpp"][:, r - ndve], t["mski"][:, r],
                                        st_t[:, si], mybir.AluOpType.mult)

        def emit_gp_mms(region, R, gp_rows, stop_last=False):
            t = tiles[region]
            for r in range(R - gp_rows, R):
                last = stop_last and r == R - 1
                mm(BASE[region] + R + r, t["gpp"][:, r - (R - gp_rows)],
                   stop=last, skip=True)

        emit_first_row()
        emit_raw("high", 4, 0, skip_first=True)
        emit_masked("high", 4, 0, 63, gp_rows=GP_OFF)
        emit_raw("low1", 8, 1)
        emit_masked("low1", 8, 1, 31, gp_rows=GP_OFF)
        emit_gp_mms("high", 4, GP_OFF)
        emit_raw("low2", 4, 2)
        emit_gp_mms("low1", 8, GP_OFF)
        emit_masked("low2", 4, 2, 31, stop_last=True)

        for ci, (o, w) in enumerate(NCHUNKS):
            # PSUM -> SBUF on the scalar (ACT) engine; frees DVE, shorter tail
            nc.scalar.copy(out_sb[:, o:o + w], psums[ci][:, :w])
            nc.sync.dma_start(out=out[:, o:o + w], in_=out_sb[:, o:o + w])

    nc.compile()
    return nc


_NC_CACHE = None


def _get_nc():
    global _NC_CACHE
    if _NC_CACHE is None:
        _NC_CACHE = build_nc()
    return _NC_CACHE


def _unpack6(packed):
    p = packed.reshape(N_HIGH // 4, 3, OUT_FEATURES)
    b0, b1, b2 = p[:, 0], p[:, 1], p[:, 2]
    v0 = b0 & 63
    v1 = ((b0 >> 6) & 3) | ((b1 & 15) << 2)
    v2 = ((b1 >> 4) & 15) | ((b2 & 3) << 4)
    v3 = (b2 >> 2) & 63
    return np.stack([v0, v1, v2, v3], axis=1).reshape(N_HIGH, OUT_FEATURES)


def _unpack5(packed):
    p = packed.reshape(N_LOW // 8, 5, OUT_FEATURES)
    b = [p[:, i] for i in range(5)]
    v0 = b[0] & 31
    v1 = ((b[0] >> 5) & 7) | ((b[1] & 3) << 3)
    v2 = (b[1] >> 2) & 31
    v3 = ((b[1] >> 7) & 1) | ((b[2] & 15) << 1)
    v4 = ((b[2] >> 4) & 15) | ((b[3] & 1) << 4)
    v5 = (b[3] >> 1) & 31
    v6 = ((b[3] >> 6) & 3) | ((b[4] & 7) << 2)
    v7 = (b[4] >> 3) & 31
    return np.stack([v0, v1, v2, v3, v4, v5, v6, v7], axis=1).reshape(
        N_LOW, OUT_FEATURES)


def _host_prep(x, W_high_packed, W_low_packed, scales_high, scales_low,
               col_indices, bias):
    """Build per-core input maps (repack weights into pair words)."""
    x = np.asarray(x, np.float32)
    Wh = np.asarray(W_high_packed, np.int32)
    Wl = np.asarray(W_low_packed, np.int32)
    sh = np.asarray(scales_high, np.float32)
    sl = np.asarray(scales_low, np.float32)
    ci = np.asarray(col_indices, np.int64)
    bias = np.asarray(bias, np.float32)

    x_perm = x[:, ci]  # [M, 4096]
    vh = _unpack6(Wh)  # [1024, N] int32, 0..63
    vl = _unpack5(Wl)  # [3072, N] int32, 0..31

    # pair words [128, R, N] int16
    k0h, k0l1, k0l2 = KMAP["high"], KMAP["low1"], KMAP["low2"]
    wh_full = (vh[k0h] | (vh[k0h + 1] << 6)).astype(np.int16)
    wl1 = (vl[k0l1] | (vl[k0l1 + 1] << 5)).astype(np.int16)
    wl2 = (vl[k0l2] | (vl[k0l2 + 1] << 5)).astype(np.int16)
    wl_full = np.concatenate([wl1, wl2], axis=1)  # [128, 12, N]

    # scale tiles [3, 128, N] bf16
    st_full = np.empty((3, 128, OUT_FEATURES), np.float32)
    st_full[0] = sh[_P // 16]
    st_full[1] = sl[_P // 8]
    st_full[2] = sl[16 + _P // 16]
    st_full = st_full.astype(BF16)

    # xdup [128, 32, M]: raw coeff = x[k1]/2^B ; masked coeff = x[k0]-x[k1]/2^B
    xdup = np.empty((128, N_BLOCKS, M), np.float32)
    xh = x_perm[:, :N_HIGH]          # [M, 1024]
    xl = x_perm[:, N_HIGH:]          # [M, 3072]
    for (xreg, k0, bi_raw, B) in [(xh, k0h, 0, 6), (xl, k0l1, 8, 5),
                                  (xl, k0l2, 24, 5)]:
        R = k0.shape[1]
        x0 = xreg[:, k0].transpose(1, 2, 0)      # [128, R, M]
        x1 = xreg[:, k0 + 1].transpose(1, 2, 0)  # [128, R, M]
        inv = 1.0 / (1 << B)
        xdup[:, bi_raw:bi_raw + R] = x1 * inv
        xdup[:, bi_raw + R:bi_raw + 2 * R] = x0 - x1 * inv
    xdup = xdup.astype(BF16)

    # correction: rows 0..31 = -h_g * group-sums of x_perm, row 32 = bias
    Xs = x_perm.reshape(M, 32, GROUP).sum(-1)  # [M, 32]
    h = np.array([31.0] * 8 + [15.0] * 24, np.float32)
    corr_lhsT = np.concatenate(
        [-(h[:, None] * Xs.T), np.ones((1, M), np.float32)], 0)  # [33, M]
    all_scales = np.concatenate([sh, sl], 0)  # [32, N]
    corr_rhs_full = np.concatenate([all_scales, bias[None]], 0)  # [33, N]

    in_maps = []
    for c in range(N_CORES):
        nsl = slice(c * N_PER, (c + 1) * N_PER)
        in_maps.append({
            "wh": np.ascontiguousarray(wh_full[..., nsl]),
            "wl": np.ascontiguousarray(wl_full[..., nsl]),
            "st": np.ascontiguousarray(st_full[..., nsl]),
            "xdup": xdup,
            "corr_lhsT": corr_lhsT,
            "corr_rhs": np.ascontiguousarray(corr_rhs_full[:, nsl]),
        })
    return in_maps


def kernel(**inputs):
    nc = _get_nc()
    in_maps = _host_prep(**inputs)
    res = run_bass_kernel_spmd(nc, in_maps, core_ids=list(range(N_CORES)))
    return np.concatenate(
        [np.asarray(r["out"]).astype(np.float32) for r in res.results], axis=1)


# revision 46
# speedup vs baseline: 1.0888x; 1.0888x over previous
# Bass/Trainium2 kernel for nn_CPRPackedLinear (mixed 6-bit/5-bit packed
# quantized linear), tensor-parallel over out_features on 8 NeuronCores.
#
# Math: out = x_perm[:, :1024] @ deq6(W_high) + x_perm[:, 1024:] @ deq5(W_low) + bias
# with deq = (unpack_bits(packed) - half) * group_scale.
#
# v2 layout ("pair packing"): the host repacks quantized values into int16
# words holding TWO same-group values (v0 | v1<<B).  Partitions are
# group-exclusive per batch, so ONE [128, N] scale tile serves every word-row
# of the batch and all DVE ops are fully batched:
#   raw plane   = words * scale          (one batched 2x tensor_tensor)
#   masked ints = words & mask           (one batched 4x tensor_scalar)
#   masked plane= masked * scale         (one batched 2x tensor_tensor)
# Value recovery is linear, folded into the host-built xdup stationary rows:
#   v1 via raw/2^B,  v0 via masked + raw cancellation.
# 32 plane rows total (= info-theoretic minimum: 4096 k / 128 partitions).
# The (-half*scale) offsets and bias enter via one small correction matmul.
import numpy as np
import ml_dtypes

import concourse.bass as bass
import concourse.mybir as mybir
from concourse import bacc
from concourse.tile import TileContext
from concourse.bass_utils import run_bass_kernel_spmd

BF16 = ml_dtypes.bfloat16

N_CORES = 8
M = 64
OUT_FEATURES = 11008
N_PER = OUT_FEATURES // N_CORES  # 1376
N_HIGH = 1024
N_LOW = 3072
GROUP = 128

NCHUNKS = [(0, 512), (512, 512), (1024, 352)]

# batches: (region, word_rows, scale_tile_idx, mask, shift)
BATCHES = [("high", 4, 0, 63, 6), ("low1", 8, 1, 31, 5), ("low2", 4, 2, 31, 5)]
N_BLOCKS = 32

_P = np.arange(128)


def _kmaps():
    """k0/k1 index maps (within-region k) for each batch, [128, R]."""
    p = _P[:, None]
    r4 = np.arange(4)[None, :]
    r8 = np.arange(8)[None, :]
    k0h = 128 * (p // 16) + (p % 16) * 8 + 2 * r4
    k0l1 = 128 * (p // 8) + (p % 8) * 16 + 2 * r8
    k0l2 = 128 * (16 + p // 16) + (p % 16) * 8 + 2 * r4
    return {"high": k0h, "low1": k0l1, "low2": k0l2}


KMAP = _kmaps()


def build_nc():
    nc = bacc.Bacc(None, name="cpr_v2", enable_partition_id=False)
    f32 = mybir.dt.float32
    bf16 = mybir.dt.bfloat16
    i16 = mybir.dt.int16

    wh = nc.dram_tensor("wh", [128, 4, N_PER], i16, kind="ExternalInput")
    wl = nc.dram_tensor("wl", [128, 12, N_PER], i16, kind="ExternalInput")
    st = nc.dram_tensor("st", [3, 128, N_PER], bf16, kind="ExternalInput")
    xd = nc.dram_tensor("xdup", [128, N_BLOCKS, M], bf16, kind="ExternalInput")
    cl = nc.dram_tensor("corr_lhsT", [33, M], f32, kind="ExternalInput")
    cr = nc.dram_tensor("corr_rhs", [33, N_PER], f32, kind="ExternalInput")
    out = nc.dram_tensor("out", [M, N_PER], bf16, kind="ExternalOutput")

    with TileContext(nc) as tc, \
         tc.tile_pool(name="const", bufs=1) as const_pool, \
         tc.tile_pool(name="words", bufs=1) as words_pool, \
         tc.tile_pool(name="planes", bufs=2) as plane_pool, \
         tc.tile_pool(name="psum", bufs=1, space="PSUM") as psum_pool:

        # small tensors + first batch's data first; the very first weight row
        # and scale tile are split at the chunk-0 boundary so the first
        # dequant op + matmul only wait on 128KB per DMA queue
        st_t = const_pool.tile([128, 3, N_PER], bf16, tag="st")
        nc.scalar.dma_start(out=st_t[:, 0, 0:512], in_=st[0][:, 0:512])
        xd_t = const_pool.tile([128, N_BLOCKS, M], bf16, tag="xd")
        nc.scalar.dma_start(out=xd_t[:, 0:4], in_=xd[:, 0:4])
        nc.scalar.dma_start(out=st_t[:, 0, 512:], in_=st[0][:, 512:])
        nc.scalar.dma_start(out=xd_t[:, 4:], in_=xd[:, 4:])
        wh_t = words_pool.tile([128, 4, N_PER], i16, tag="wh")
        nc.sync.dma_start(out=wh_t[:, 0:1, 0:512], in_=wh[:, 0:1, 0:512])
        nc.sync.dma_start(out=wh_t[:, 0:1, 512:], in_=wh[:, 0:1, 512:])
        nc.sync.dma_start(out=wh_t[:, 1:2], in_=wh[:, 1:2])
        nc.sync.dma_start(out=wh_t[:, 2:4], in_=wh[:, 2:4])
        cl_t = const_pool.tile([33, M], f32, tag="cl")
        nc.scalar.dma_start(out=cl_t[:], in_=cl[:])
        cr_t = const_pool.tile([33, N_PER], f32, tag="cr")
        nc.scalar.dma_start(out=cr_t[:], in_=cr[:])
        nc.scalar.dma_start(out=st_t[:, 1], in_=st[1])
        wl_t = words_pool.tile([128, 12, N_PER], i16, tag="wl")
        for i in range(0, 8, 2):
            nc.sync.dma_start(out=wl_t[:, i:i + 2], in_=wl[:, i:i + 2])
        nc.scalar.dma_start(out=st_t[:, 2], in_=st[2])
        nc.sync.dma_start(out=wl_t[:, 8:10], in_=wl[:, 8:10])
        nc.sync.dma_start(out=wl_t[:, 10:12], in_=wl[:, 10:12])

        out_sb = const_pool.tile([M, N_PER], bf16, tag="outsb")
        psums = [psum_pool.tile([M, w], mybir.dt.float32, tag=f"ps{i}",
                                name=f"ps{i}")
                 for i, (o, w) in enumerate(NCHUNKS)]

        def mm(bi, plane_ap, start=False, stop=False, skip=False):
            for ci, (o, w) in enumerate(NCHUNKS):
                nc.tensor.matmul(
                    psums[ci][:, :w], xd_t[:, bi, :], plane_ap[:, o:o + w],
                    start=start, stop=stop, skip_group_check=skip,
                )
            if start:
                # correction accumulates early (order-free in PSUM)
                for ci, (o, w) in enumerate(NCHUNKS):
                    nc.tensor.matmul(
                        psums[ci][:, :w], cl_t[:], cr_t[:, o:o + w],
                        start=False, stop=False, skip_group_check=True,
                    )

        # rows per DVE op: small first piece starts the PE early, bigger
        # pieces amortize per-op overhead afterwards.  The last GP_OFF rows
        # of the high-masked and low2-masked blocks run on GPSIMD (slower,
        # but in parallel with the DVE); their matmuls are issued late so
        # the strict-FIFO PE queue never blocks waiting on GPSIMD.
        # raw high pieces cover rows 1..3 (row 0 is emitted column-split);
        # masked blocks use MPIECES (all rows)
        PIECES = {"high": [1, 2], "low1": [4, 4], "low2": [2, 1, 1]}
        MPIECES = {"high": [2, 2], "low1": [4, 4], "low2": [2, 1, 1]}
        GP_OFF = 0
        BASE = {"high": 0, "low1": 8, "low2": 24}

        tiles = {}
        for (region, R, si, mask, shift) in BATCHES:
            if region == "high":
                words = wh_t[:]
            elif region == "low1":
                words = wl_t[:, 0:8]
            else:
                words = wl_t[:, 8:12]
            tiles[region] = dict(
                words=words, si=si,
                raw=plane_pool.tile([128, R, N_PER], bf16, tag="raw",
                                    name=f"raw{si}", bufs=2),
                mski=plane_pool.tile([128, R, N_PER], i16, tag="mski",
                                     name=f"mski{si}", bufs=2),
                mskp=plane_pool.tile([128, R, N_PER], bf16, tag="mskp",
                                     name=f"mskp{si}", bufs=2),
                gpp=(plane_pool.tile([128, GP_OFF, N_PER], bf16, tag="gpp",
                                     name=f"gpp{si}", bufs=2)
                     if GP_OFF else None),
            )

        def stb(si, rows):
            return st_t[:, si].rearrange("p (o n) -> p o n", o=1) \
                              .broadcast_to([128, rows, N_PER])

        def emit_first_row():
            """High row 0, column-split at the chunk-0 edge: the first matmul
            (chunk 0, start + correction) launches off a quarter of the row."""
            t = tiles["high"]
            for (c0, c1) in [(0, 512), (512, N_PER)]:
                nc.vector.tensor_tensor(
                    t["raw"][:, 0:1, c0:c1], t["words"][:, 0:1, c0:c1],
                    st_t[:, 0, c0:c1].rearrange("p (o n) -> p o n", o=1),
                    mybir.AluOpType.mult)
                for ci, (o, w) in enumerate(NCHUNKS):
                    if not (c0 <= o < c1):
                        continue
                    nc.tensor.matmul(
                        psums[ci][:, :w], xd_t[:, 0, :], t["raw"][:, 0, o:o + w],
                        start=True, stop=False)

        def emit_corr():
            for ci, (o, w) in enumerate(NCHUNKS):
                nc.tensor.matmul(
                    psums[ci][:, :w], cl_t[:], cr_t[:, o:o + w],
                    start=False, stop=False, skip_group_check=True)

        def emit_raw(region, R, si, skip_first=False):
            t = tiles[region]
            r0 = 1 if skip_first else 0
            for pc in PIECES[region]:
                sl_ = slice(r0, r0 + pc)
                nc.vector.tensor_tensor(t["raw"][:, sl_], t["words"][:, sl_],
                                        stb(si, pc), mybir.AluOpType.mult)
                for r in range(r0, r0 + pc):
                    mm(BASE[region] + r, t["raw"][:, r])
                r0 += pc

        def emit_masked(region, R, si, mask, gp_rows=0, mm_skip_gp=True,
                        stop_last=False):
            """DVE part of the masked block; last gp_rows rows go to GPSIMD
            (compute only - their matmuls are emitted by emit_gp_mms)."""
            t = tiles[region]
            ndve = R - gp_rows
            r0 = 0
            for pc in MPIECES[region]:
                sl_ = slice(r0, r0 + pc)
                nc.vector.tensor_scalar(t["mski"][:, sl_], t["words"][:, sl_],
                                        mask, None,
                                        mybir.AluOpType.bitwise_and)
                if r0 < ndve:
                    dv = slice(r0, min(r0 + pc, ndve))
                    nc.vector.tensor_tensor(t["mskp"][:, dv], t["mski"][:, dv],
                                            stb(si, dv.stop - dv.start),
                                            mybir.AluOpType.mult)
                    for r in range(dv.start, dv.stop):
                        last = stop_last and gp_rows == 0 and r == R - 1
                        mm(BASE[region] + R + r, t["mskp"][:, r], stop=last)
                r0 += pc
            for r in range(ndve, R):
                nc.gpsimd.tensor_tensor(t["gpp"][:, r - ndve], t["mski"][:, r],
                                        st_t[:, si], mybir.AluOpType.mult)

        def emit_gp_mms(region, R, gp_rows, stop_last=False):
            t = tiles[region]
            for r in range(R - gp_rows, R):
                last = stop_last and r == R - 1
                mm(BASE[region] + R + r, t["gpp"][:, r - (R - gp_rows)],
                   stop=last, skip=True)

        emit_first_row()
        emit_raw("high", 4, 0, skip_first=True)
        emit_corr()
        emit_masked("high", 4, 0, 63, gp_rows=GP_OFF)
        emit_raw("low1", 8, 1)
        emit_masked("low1", 8, 1, 31, gp_rows=GP_OFF)
        emit_gp_mms("high", 4, GP_OFF)
        emit_raw("low2", 4, 2)
        emit_gp_mms("low1", 8, GP_OFF)
        emit_masked("low2", 4, 2, 31, stop_last=True)

        for ci, (o, w) in enumerate(NCHUNKS):
            # PSUM -> SBUF on the scalar (ACT) engine; frees DVE, shorter tail
            nc.scalar.copy(out_sb[:, o:o + w], psums[ci][:, :w])
            nc.sync.dma_start(out=out[:, o:o + w], in_=out_sb[:, o:o + w])

    nc.compile()
    return nc


_NC_CACHE = None


def _get_nc():
    global _NC_CACHE
    if _NC_CACHE is None:
        _NC_CACHE = build_nc()
    return _NC_CACHE


def _unpack6(packed):
    p = packed.reshape(N_HIGH // 4, 3, OUT_FEATURES)
    b0, b1, b2 = p[:, 0], p[:, 1], p[:, 2]
    v0 = b0 & 63
    v1 = ((b0 >> 6) & 3) | ((b1 & 15) << 2)
    v2 = ((b1 >> 4) & 15) | ((b2 & 3) << 4)
    v3 = (b2 >> 2) & 63
    return np.stack([v0, v1, v2, v3], axis=1).reshape(N_HIGH, OUT_FEATURES)


def _unpack5(packed):
    p = packed.reshape(N_LOW // 8, 5, OUT_FEATURES)
    b = [p[:, i] for i in range(5)]
    v0 = b[0] & 31
    v1 = ((b[0] >> 5) & 7) | ((b[1] & 3) << 3)
    v2 = (b[1] >> 2) & 31
    v3 = ((b[1] >> 7) & 1) | ((b[2] & 15) << 1)
    v4 = ((b[2] >> 4) & 15) | ((b[3] & 1) << 4)
    v5 = (b[3] >> 1) & 31
    v6 = ((b[3] >> 6) & 3) | ((b[4] & 7) << 2)
    v7 = (b[4] >> 3) & 31
    return np.stack([v0, v1, v2, v3, v4, v5, v6, v7], axis=1).reshape(
        N_LOW, OUT_FEATURES)


def _host_prep(x, W_high_packed, W_low_packed, scales_high, scales_low,
               col_indices, bias):
    """Build per-core input maps (repack weights into pair words)."""
    x = np.asarray(x, np.float32)
    Wh = np.asarray(W_high_packed, np.int32)
    Wl = np.asarray(W_low_packed, np.int32)
    sh = np.asarray(scales_high, np.float32)
    sl = np.asarray(scales_low, np.float32)
    ci = np.asarray(col_indices, np.int64)
    bias = np.asarray(bias, np.float32)

    x_perm = x[:, ci]  # [M, 4096]
    vh = _unpack6(Wh)  # [1024, N] int32, 0..63
    vl = _unpack5(Wl)  # [3072, N] int32, 0..31

    # pair words [128, R, N] int16
    k0h, k0l1, k0l2 = KMAP["high"], KMAP["low1"], KMAP["low2"]
    wh_full = (vh[k0h] | (vh[k0h + 1] << 6)).astype(np.int16)
    wl1 = (vl[k0l1] | (vl[k0l1 + 1] << 5)).astype(np.int16)
    wl2 = (vl[k0l2] | (vl[k0l2 + 1] << 5)).astype(np.int16)
    wl_full = np.concatenate([wl1, wl2], axis=1)  # [128, 12, N]

    # scale tiles [3, 128, N] bf16
    st_full = np.empty((3, 128, OUT_FEATURES), np.float32)
    st_full[0] = sh[_P // 16]
    st_full[1] = sl[_P // 8]
    st_full[2] = sl[16 + _P // 16]
    st_full = st_full.astype(BF16)

    # xdup [128, 32, M]: raw coeff = x[k1]/2^B ; masked coeff = x[k0]-x[k1]/2^B
    xdup = np.empty((128, N_BLOCKS, M), np.float32)
    xh = x_perm[:, :N_HIGH]          # [M, 1024]
    xl = x_perm[:, N_HIGH:]          # [M, 3072]
    for (xreg, k0, bi_raw, B) in [(xh, k0h, 0, 6), (xl, k0l1, 8, 5),
                                  (xl, k0l2, 24, 5)]:
        R = k0.shape[1]
        x0 = xreg[:, k0].transpose(1, 2, 0)      # [128, R, M]
        x1 = xreg[:, k0 + 1].transpose(1, 2, 0)  # [128, R, M]
        inv = 1.0 / (1 << B)
        xdup[:, bi_raw:bi_raw + R] = x1 * inv
        xdup[:, bi_raw + R:bi_raw + 2 * R] = x0 - x1 * inv
    xdup = xdup.astype(BF16)

    # correction: rows 0..31 = -h_g * group-sums of x_perm, row 32 = bias
    Xs = x_perm.reshape(M, 32, GROUP).sum(-1)  # [M, 32]
    h = np.array([31.0] * 8 + [15.0] * 24, np.float32)
    corr_lhsT = np.concatenate(
        [-(h[:, None] * Xs.T), np.ones((1, M), np.float32)], 0)  # [33, M]
    all_scales = np.concatenate([sh, sl], 0)  # [32, N]
    corr_rhs_full = np.concatenate([all_scales, bias[None]], 0)  # [33, N]

    in_maps = []
    for c in range(N_CORES):
        nsl = slice(c * N_PER, (c + 1) * N_PER)
        in_maps.append({
            "wh": np.ascontiguousarray(wh_full[..., nsl]),
            "wl": np.ascontiguousarray(wl_full[..., nsl]),
            "st": np.ascontiguousarray(st_full[..., nsl]),
            "xdup": xdup,
            "corr_lhsT": corr_lhsT,
            "corr_rhs": np.ascontiguousarray(corr_rhs_full[:, nsl]),
        })
    return in_maps


def kernel(**inputs):
    nc = _get_nc()
    in_maps = _host_prep(**inputs)
    res = run_bass_kernel_spmd(nc, in_maps, core_ids=list(range(N_CORES)))
    return np.concatenate(
        [np.asarray(r["out"]).astype(np.float32) for r in res.results], axis=1)
